# revision 16
# baseline (speedup 1.0000x reference)
"""Trainium2 Bass kernel for nn_DivTree (moe_routing) — bf16, preloaded
weights, batch-stationary L3, partition-major DRAM layouts.

Computation (per reference):
    x1 = relu(x0 @ W_shared + b_shared)         # [B, A, H]
    h  = relu(einsum('bah,ahk', x1, W1[route]) + b1[route])
    y  = einsum('bah,ahk', h, W2[route]) + b2[route]   # [B, A, NA]

Strategy: data-parallel over batch across 8 NeuronCores (512 rows/core),
weights replicated, agents grouped by expert (8 distinct experts).
Feature-major layout for L1/L2: contraction on SBUF partitions, weights
stationary, batch as the 512-wide moving free dim. All matmul operands
bf16 (fp32 PSUM accumulation, fp32 output).

Measured-on-HW design notes (microbench + trace, this session):
  * A 512-free bf16 matmul takes ~216ns at full clock (213.3 ideal) —
    PE-roofline-bound; run-to-run DVFS moves this to 235-260ns.
  * LDWEIGHTS fully overlaps the previous matmul (97ns slices in its
    shadow); stationary reloads are free.
  * L3 batch-stationary — stationary = h [128k, 128b], moving = W2
    [128k, 32] — runs at ~26ns/matmul: 16 tiny matmuls (~0.5us)
    replace 4 big ones (~0.94us) per agent: −14us of PE time.
  * ALL expert weights (4.5MB bf16) are preloaded into SBUF (34KB of
    the 208KB partition budget): steady state has zero weight DMAs.
  * DMA triggers cost ~8ns PER DESCRIPTOR on the issuing queue
    (DIRECT2D slices): a strided rearrange like "e (ms p) -> p e ms"
    is a descriptor bomb (4096 descs = 32us of queue time). Every
    DRAM tensor is therefore pre-transposed on the host so each
    transfer is one contiguous chunk per partition (128 descriptors,
    ~1us of queue time), including the output stores.
  * fp8 DoubleRow doubles PE throughput but e4m3 quantization of even
    ONE layer gives 3.5e-2 Frobenius error vs the 2e-2 gate — dead.

Pipeline: two-stage software pipeline over agent pairs, L3 of agents
t-4,t-3 deferred until after L1 of the pair t,t+1 so the PE never waits
on h activations; output stores are deferred one iteration so their
queue-blocking wait never delays x0 prefetch triggers. Head DMAs are
spread across the three DMA-capable queues (sync/gpsimd/scalar) with
the first L1 chain's operands split piecewise across all three.
"""

import numpy as np

P = 128
N_CORES = 8
WARMUP = 18

_cache: dict = {}


def _build(A, D, H, NA, Bl, groups):
    import concourse.mybir as mybir
    import concourse.tile as tile
    from concourse import bacc
    from contextlib import ExitStack

    f32 = mybir.dt.float32
    bf16 = mybir.dt.bfloat16
    Relu = mybir.ActivationFunctionType.Relu
    E = len(groups)
    KD, KH, MH = D // P, H // P, H // P
    NB = Bl  # matmul free dim (batch); Bl=512 fits one PSUM bank
    JB = NB // P  # batch blocks of 128 (stationary columns in L3)
    assert NB <= 512 and H % P == 0 and D % P == 0 and NA <= P

    agent_list = [(s, a) for s, agents in enumerate(groups) for a in agents]
    NAG = len(agent_list)

    nc = bacc.Bacc()
    # partition-major layouts: every DMA moves one contiguous chunk per
    # partition (see module docstring)
    x0p = nc.declare_dram_parameter("x0p", [A, P, KD, Bl], bf16,
                                    isOutput=False)
    wsp = nc.declare_dram_parameter("wsp", [P, MH, KD, P], bf16,
                                    isOutput=False)
    bsp = nc.declare_dram_parameter("bsp", [P, MH], f32, isOutput=False)
    w1p = nc.declare_dram_parameter("w1p", [E, P, KH, H], bf16,
                                    isOutput=False)
    b1p = nc.declare_dram_parameter("b1p", [P, E, MH], f32, isOutput=False)
    w2p = nc.declare_dram_parameter("w2p", [P, E, KH, NA], bf16,
                                    isOutput=False)
    b2p = nc.declare_dram_parameter("b2p", [P, E, NA], f32, isOutput=False)
    yt = nc.declare_dram_parameter("yt", [A, P, JB, NA], f32, isOutput=True)

    with tile.TileContext(nc) as tc, ExitStack() as ctx:
        const = ctx.enter_context(tc.tile_pool(name="const", bufs=1))
        xpool = ctx.enter_context(tc.tile_pool(name="x0", bufs=5))
        x1pool = ctx.enter_context(tc.tile_pool(name="x1", bufs=5))
        hpool = ctx.enter_context(tc.tile_pool(name="h", bufs=5))
        opool = ctx.enter_context(tc.tile_pool(name="out", bufs=3))
        psum = ctx.enter_context(tc.tile_pool(name="ps", bufs=3, space="PSUM"))
        psum2 = ctx.enter_context(tc.tile_pool(name="ps2", bufs=3, space="PSUM"))
        psum3 = ctx.enter_context(tc.tile_pool(name="ps3", bufs=2, space="PSUM"))

        # PE warm-up: the HAM clock gate holds the array at low clock until
        # it has been busy a while; burn dummy matmuls during the initial
        # DMA wait so real matmuls start at a higher clock.
        dummy = const.tile([P, 128], bf16)
        nc.gpsimd.memset(dummy[:], 0.0)
        dummy2 = const.tile([P, 512], bf16)
        nc.gpsimd.memset(dummy2[:], 0.0)
        dps = psum.tile([64, 512], f32, tag="ps")
        for i in range(WARMUP):
            nc.tensor.matmul(dps[:, :128], dummy[:, :64], dummy[:, :128],
                             start=True, stop=True)
        # longer dummies sized so the warmup ends right as the first x0/wsm
        # pieces land (~11.5us); each is ~427ns at the ramp clock
        for i in range(5):
            nc.tensor.matmul(dps[:], dummy2[:, :64], dummy2[:],
                             start=True, stop=True)

        # ---- head DMA issue, spread across the 3 trigger queues ----
        # each trigger costs ~610ns of queue time, so the first L1 chain's
        # operands (x0[a0] k-pieces, wsm0) take the FIRST two slots of
        # every queue; everything else follows by deadline
        a0 = agent_list[0][1]
        x0_first = xpool.tile([P, KD, NB], bf16, tag="x0")
        wsm = [const.tile([P, KD, P], bf16, tag=f"wsm{ms}", name=f"wsm{ms}")
               for ms in range(MH)]
        # a transfer's descriptors ride ONE DMA ring (~23-46GB/s), so the
        # first chain's x0 k-pieces are split into halves that pull on
        # parallel rings; queue slot order is by consumption deadline
        HB = NB // 2
        q0, q1, q2 = nc.sync, nc.gpsimd, nc.scalar
        q0.dma_start(x0_first[:, 0, :HB], x0p[a0][:, 0, :HB])
        q1.dma_start(x0_first[:, 0, HB:], x0p[a0][:, 0, HB:])
        q2.dma_start(x0_first[:, 1, :HB], x0p[a0][:, 1, :HB])
        q0.dma_start(x0_first[:, 1, HB:], x0p[a0][:, 1, HB:])
        q1.dma_start(wsm[0][:, 0, :], wsp[:, 0, 0, :])
        q2.dma_start(wsm[0][:, 1, :], wsp[:, 0, 1, :])
        q1.dma_start(x0_first[:, 2, :HB], x0p[a0][:, 2, :HB])
        q2.dma_start(x0_first[:, 2, HB:], x0p[a0][:, 2, HB:])
        q0.dma_start(wsm[0][:, 2, :], wsp[:, 0, 2, :])
        q0.dma_start(x0_first[:, 3, :HB], x0p[a0][:, 3, :HB])
        q2.dma_start(x0_first[:, 3, HB:], x0p[a0][:, 3, HB:])
        q1.dma_start(wsm[0][:, 3, :], wsp[:, 0, 3, :])
        q1.dma_start(wsm[1][:], wsp[:, 1])
        q2.dma_start(wsm[2][:], wsp[:, 2])
        q0.dma_start(wsm[3][:], wsp[:, 3])
        bs_t = const.tile([P, MH], f32)
        nc.scalar.dma_start(bs_t[:], bsp[:, :])

        # preloaded expert weights: all groups resident in SBUF
        w1all = const.tile([P, E, KH, H], bf16, name="w1all")
        w2all = const.tile([P, E, KH, NA], bf16, name="w2all")
        b1all = const.tile([P, E, MH], f32, name="b1all")
        b2all = const.tile([P, E, NA], f32, name="b2all")

        x0_tiles = {0: x0_first}

        def dma_x0(t, eng, eng2=None):
            if t >= NAG or t in x0_tiles:
                return
            a = agent_list[t][1]
            x0_t = xpool.tile([P, KD, NB], bf16, tag="x0", name=f"x0_{a}")
            if eng2 is not None:  # head: split halves across two queues
                eng.dma_start(x0_t[:, :KD // 2], x0p[a][:, :KD // 2])
                eng2.dma_start(x0_t[:, KD // 2:], x0p[a][:, KD // 2:])
            else:
                eng.dma_start(x0_t[:], x0p[a])
            x0_tiles[t] = x0_t

        dma_x0(1, nc.gpsimd, nc.scalar)
        dma_x0(2, nc.gpsimd, nc.scalar)
        # group 0's W1 is on the first-L2 critical path (~22us); halves so
        # both rings pull it in parallel
        nc.gpsimd.dma_start(w1all[:, 0, :KH // 2], w1p[0][:, :KH // 2])
        nc.scalar.dma_start(w1all[:, 0, KH // 2:], w1p[0][:, KH // 2:])
        nc.sync.dma_start(b1all[:], b1p[:, :])
        nc.sync.dma_start(b2all[:], b2p[:, :])
        nc.sync.dma_start(w2all[:], w2p[:, :])
        # W1 for groups 1..E-1 (4MB total) is staggered into the agent
        # loop — one group per pair — so the head's x0 transfers don't
        # fight it for DMA-ring bandwidth; group s is first needed around
        # 27us*s with the trigger fired ~20us*s, transfer ~5us.
        w1_pending = list(range(1, E))

        # steady-state x0 prefetch: one trigger, rotating queues (sync is
        # reserved for output stores)
        dma_engines = [nc.gpsimd, nc.scalar]

        def emit_l1(a, x0_t):
            x1_t = x1pool.tile([P, MH, NB], bf16, tag="x1", name=f"x1_{a}")
            for ms in range(MH):
                ps1 = psum.tile([P, NB], f32, tag="ps", name=f"ps1_{a}_{ms}")
                for ks in range(KD):
                    nc.tensor.matmul(
                        ps1[:], wsm[ms][:, ks, :], x0_t[:, ks, :],
                        start=(ks == 0), stop=(ks == KD - 1),
                    )
                if ms % 2:
                    nc.vector.tensor_scalar(
                        x1_t[:, ms, :], ps1[:], bs_t[:, ms:ms + 1], 0.0,
                        mybir.AluOpType.add, mybir.AluOpType.max)
                else:
                    nc.scalar.activation(x1_t[:, ms, :], ps1[:], Relu,
                                         bias=bs_t[:, ms:ms + 1])
            return x1_t

        def emit_l2(a, x1_t, s):
            h_t = hpool.tile([P, MH, NB], bf16, tag="h", name=f"h_{a}")
            for ms in range(MH):
                ps2 = psum2.tile([P, NB], f32, tag="ps2", name=f"ps2_{a}_{ms}")
                for ks in range(KH):
                    nc.tensor.matmul(
                        ps2[:],
                        w1all[:, s, ks, ms * P:(ms + 1) * P],
                        x1_t[:, ks, :],
                        start=(ks == 0), stop=(ks == KH - 1),
                    )
                if ms % 2:
                    nc.vector.tensor_scalar(
                        h_t[:, ms, :], ps2[:], b1all[:, s, ms:ms + 1], 0.0,
                        mybir.AluOpType.add, mybir.AluOpType.max)
                else:
                    nc.scalar.activation(h_t[:, ms, :], ps2[:], Relu,
                                         bias=b1all[:, s, ms:ms + 1])
            return h_t

        def emit_l3_tail(a, h_t, s):
            # batch-stationary: stationary = h [128k, 128b-block], moving =
            # W2 [128k, NA]; out [128b, NA] accumulated over KH k-tiles.
            ps3 = psum3.tile([P, JB, NA], f32, tag="ps3", name=f"ps3_{a}")
            for jb in range(JB):
                for ks in range(KH):
                    nc.tensor.matmul(
                        ps3[:, jb, :],
                        h_t[:, ks, jb * P:(jb + 1) * P],
                        w2all[:, s, ks, :],
                        start=(ks == 0), stop=(ks == KH - 1),
                    )
            o_t = opool.tile([P, JB, NA], f32, tag="o", name=f"o_{a}")
            nc.vector.tensor_add(
                o_t[:], ps3[:],
                b2all[:, s:s + 1, :].to_broadcast((P, JB, NA)),
            )
            # store trigger deferred to the NEXT iteration: on the sync
            # queue it blocks until the DVE add completes, and anything
            # queued behind it would inherit that wait
            return (a, o_t)

        def store(a, o_t):
            nc.sync.dma_start(yt[a], o_t[:])

        # two-stage software pipeline over agent PAIRS: each round emits
        #   L1(a), L1(a+1) | L3(a-4), L3(a-3) | L2(a-2), L2(a-1)
        pend_l2 = []     # [(a, x1_t, s)] — L1 done, L2 not yet emitted
        pend_tail = []   # [(a, h_t, s)] — L2 done, L3 deferred
        pend_store = []  # [(a, o_t)] — output computed, store not queued
        for t, (s, a) in enumerate(agent_list):
            dma_x0(t + 3, dma_engines[t % 2])
            if t % 2 == 0 and w1_pending:
                sw = w1_pending.pop(0)
                nc.sync.dma_start(w1all[:, sw], w1p[sw])
            for sa, so in pend_store:
                store(sa, so)
            pend_store = []
            x1_t = emit_l1(a, x0_tiles.pop(t))
            pend_l2.append((a, x1_t, s))
            if t % 2 == 1:
                for args in pend_tail:
                    pend_store.append(emit_l3_tail(*args))
                pend_tail = []
                while len(pend_l2) > 2:
                    pa, px1, ps_ = pend_l2.pop(0)
                    h_t = emit_l2(pa, px1, ps_)
                    pend_tail.append((pa, h_t, ps_))
        def emit_l3_final(a, h_t, s, engs):
            # tail-only variant: per-half bias add + store on separate
            # queues/rings so the final 64KB store (~2.8us on one ring)
            # doesn't serialize after the last matmul
            JH = JB // 2
            o_t = opool.tile([P, JB, NA], f32, tag="o", name=f"of_{a}")
            for half in range(2):
                ps3 = psum3.tile([P, JH, NA], f32, tag="ps3",
                                 name=f"ps3f_{a}_{half}")
                for jj in range(JH):
                    jb = half * JH + jj
                    for ks in range(KH):
                        nc.tensor.matmul(
                            ps3[:, jj, :],
                            h_t[:, ks, jb * P:(jb + 1) * P],
                            w2all[:, s, ks, :],
                            start=(ks == 0), stop=(ks == KH - 1),
                        )
                sl = slice(half * JH, (half + 1) * JH)
                nc.vector.tensor_add(
                    o_t[:, sl], ps3[:],
                    b2all[:, s:s + 1, :].to_broadcast((P, JH, NA)),
                )
                engs[half].dma_start(yt[a][:, sl], o_t[:, sl])

        # drain — both remaining L2s before both L3 tails, so the first
        # tail's activations get a full L2 phase of slack
        for sa, so in pend_store:
            store(sa, so)
        for args in pend_tail:
            store(*emit_l3_tail(*args))
        done_l2 = []
        for pa, px1, ps_ in pend_l2:
            done_l2.append((pa, emit_l2(pa, px1, ps_), ps_))
        tail_engs = [(nc.sync, nc.gpsimd), (nc.scalar, nc.gpsimd)]
        for i, (pa, h_t, ps_) in enumerate(done_l2):
            emit_l3_final(pa, h_t, ps_, tail_engs[i % 2])

    nc.compile()
    return nc


def kernel(x0, W_shared, b_shared, W1, b1, W2, b2, route,
           _trace=False, _tmpdir=None):
    import ml_dtypes
    from concourse.bass_utils import run_bass_kernel_spmd

    bf16 = ml_dtypes.bfloat16
    x0 = np.asarray(x0, dtype=np.float32)
    W_shared = np.asarray(W_shared, dtype=np.float32)
    b_shared = np.asarray(b_shared, dtype=np.float32)
    W1 = np.asarray(W1, dtype=np.float32)
    b1 = np.asarray(b1, dtype=np.float32)
    W2 = np.asarray(W2, dtype=np.float32)
    b2 = np.asarray(b2, dtype=np.float32)
    route = np.asarray(route)

    B, A, D = x0.shape
    H = W_shared.shape[1]
    NA = W2.shape[2]
    Bl = B // N_CORES
    KD, KH, MH = D // P, H // P, H // P
    JB = Bl // P

    experts, inv = np.unique(route, return_inverse=True)
    groups = tuple(tuple(np.where(inv == s)[0].tolist())
                   for s in range(len(experts)))
    E = len(experts)

    key = (B, A, D, H, NA, groups)
    nc = _cache.get(key)
    if nc is None:
        nc = _build(A, D, H, NA, Bl, groups)
        _cache[key] = nc

    # host-side shard + transpose to partition-major bf16 layouts
    # x0p[c][a, p, ks, b] = x0[c*Bl + b, a, ks*P + p]
    x0p = np.ascontiguousarray(
        x0.astype(bf16).reshape(N_CORES, Bl, A, KD, P)
        .transpose(0, 2, 4, 3, 1))
    # wsp[p, ms, ks, q] = Ws[ks*P + p, ms*P + q]
    wsp = np.ascontiguousarray(
        W_shared.astype(bf16).reshape(KD, P, MH, P).transpose(1, 2, 0, 3))
    bsp = np.ascontiguousarray(b_shared.reshape(MH, P).T)
    # w1p[s, p, ks, h] = W1[experts[s], ks*P + p, h]
    w1p = np.ascontiguousarray(
        W1[experts].astype(bf16).reshape(E, KH, P, H).transpose(0, 2, 1, 3))
    # b1p[p, s, ms] = b1[experts[s], ms*P + p]
    b1p = np.ascontiguousarray(
        b1[experts].reshape(E, MH, P).transpose(2, 0, 1))
    # w2p[p, s, ks, n] = W2[experts[s], ks*P + p, n]
    w2p = np.ascontiguousarray(
        W2[experts].astype(bf16).reshape(E, KH, P, NA).transpose(2, 0, 1, 3))
    b2p = np.ascontiguousarray(
        np.broadcast_to(b2[experts][None, :, :], (P, E, NA)))

    in_maps = [
        dict(x0p=x0p[c], wsp=wsp, bsp=bsp,
             w1p=w1p, b1p=b1p, w2p=w2p, b2p=b2p)
        for c in range(N_CORES)
    ]
    # the axon-proxied runtime occasionally reports a transient
    # "device unrecoverable" right after another process released the
    # cores; a short-delay retry recovers it
    import time
    last_err = None
    for attempt in range(3):
        try:
            res = run_bass_kernel_spmd(nc, in_maps,
                                       core_ids=list(range(N_CORES)),
                                       trace=_trace, tmpdir=_tmpdir)
            break
        except Exception as e:  # noqa: BLE001
            last_err = e
            time.sleep(5.0 * (attempt + 1))
    else:
        raise last_err
    kernel.last_exec_time_ns = res.exec_time_ns
    # yt[c][a, p, j, n] = y[c*Bl + j*P + p, a, n]
    yt = np.stack([res.results[c]["yt"] for c in range(N_CORES)])
    y = np.ascontiguousarray(
        yt.transpose(0, 3, 2, 1, 4)).reshape(B, A, NA)
    return y


# revision 20
# speedup vs baseline: 1.0045x; 1.0045x over previous
"""Trainium2 Bass kernel for nn_DivTree (moe_routing) — bf16, preloaded
weights, batch-stationary L3, partition-major DRAM layouts.

Computation (per reference):
    x1 = relu(x0 @ W_shared + b_shared)         # [B, A, H]
    h  = relu(einsum('bah,ahk', x1, W1[route]) + b1[route])
    y  = einsum('bah,ahk', h, W2[route]) + b2[route]   # [B, A, NA]

Strategy: data-parallel over batch across 8 NeuronCores (512 rows/core),
weights replicated, agents grouped by expert (8 distinct experts).
Feature-major layout for L1/L2: contraction on SBUF partitions, weights
stationary, batch as the 512-wide moving free dim. All matmul operands
bf16 (fp32 PSUM accumulation, fp32 output).

Measured-on-HW design notes (microbench + trace, this session):
  * A 512-free bf16 matmul takes ~216ns at full clock (213.3 ideal) —
    PE-roofline-bound; run-to-run DVFS moves this to 235-260ns.
  * LDWEIGHTS fully overlaps the previous matmul (97ns slices in its
    shadow); stationary reloads are free.
  * L3 batch-stationary — stationary = h [128k, 128b], moving = W2
    [128k, 32] — runs at ~26ns/matmul: 16 tiny matmuls (~0.5us)
    replace 4 big ones (~0.94us) per agent: −14us of PE time.
  * ALL expert weights (4.5MB bf16) are preloaded into SBUF (34KB of
    the 208KB partition budget): steady state has zero weight DMAs.
  * DMA triggers cost ~8ns PER DESCRIPTOR on the issuing queue
    (DIRECT2D slices): a strided rearrange like "e (ms p) -> p e ms"
    is a descriptor bomb (4096 descs = 32us of queue time). Every
    DRAM tensor is therefore pre-transposed on the host so each
    transfer is one contiguous chunk per partition (128 descriptors,
    ~1us of queue time), including the output stores.
  * fp8 DoubleRow doubles PE throughput but e4m3 quantization of even
    ONE layer gives 3.5e-2 Frobenius error vs the 2e-2 gate — dead.

Pipeline: two-stage software pipeline over agent pairs, L3 of agents
t-4,t-3 deferred until after L1 of the pair t,t+1 so the PE never waits
on h activations; output stores are deferred one iteration so their
queue-blocking wait never delays x0 prefetch triggers. Head DMAs are
spread across the three DMA-capable queues (sync/gpsimd/scalar) with
the first L1 chain's operands split piecewise across all three.
"""

import numpy as np

P = 128
N_CORES = 8
WARMUP = 18

_cache: dict = {}


def _build(A, D, H, NA, Bl, groups):
    import concourse.mybir as mybir
    import concourse.tile as tile
    from concourse import bacc
    from contextlib import ExitStack

    f32 = mybir.dt.float32
    bf16 = mybir.dt.bfloat16
    Relu = mybir.ActivationFunctionType.Relu
    E = len(groups)
    KD, KH, MH = D // P, H // P, H // P
    NB = Bl  # matmul free dim (batch); Bl=512 fits one PSUM bank
    JB = NB // P  # batch blocks of 128 (stationary columns in L3)
    assert NB <= 512 and H % P == 0 and D % P == 0 and NA <= P

    agent_list = [(s, a) for s, agents in enumerate(groups) for a in agents]
    NAG = len(agent_list)

    nc = bacc.Bacc()
    # partition-major layouts: every DMA moves one contiguous chunk per
    # partition (see module docstring)
    x0p = nc.declare_dram_parameter("x0p", [A, P, KD, Bl], bf16,
                                    isOutput=False)
    wsp = nc.declare_dram_parameter("wsp", [P, MH, KD, P], bf16,
                                    isOutput=False)
    bsp = nc.declare_dram_parameter("bsp", [P, MH], f32, isOutput=False)
    w1p = nc.declare_dram_parameter("w1p", [E, P, KH, H], bf16,
                                    isOutput=False)
    b1p = nc.declare_dram_parameter("b1p", [P, E, MH], f32, isOutput=False)
    w2p = nc.declare_dram_parameter("w2p", [P, E, KH, NA], bf16,
                                    isOutput=False)
    b2p = nc.declare_dram_parameter("b2p", [P, E, NA], f32, isOutput=False)
    yt = nc.declare_dram_parameter("yt", [A, P, JB, NA], f32, isOutput=True)

    with tile.TileContext(nc) as tc, ExitStack() as ctx:
        const = ctx.enter_context(tc.tile_pool(name="const", bufs=1))
        xpool = ctx.enter_context(tc.tile_pool(name="x0", bufs=5))
        x1pool = ctx.enter_context(tc.tile_pool(name="x1", bufs=5))
        hpool = ctx.enter_context(tc.tile_pool(name="h", bufs=5))
        opool = ctx.enter_context(tc.tile_pool(name="out", bufs=3))
        psum = ctx.enter_context(tc.tile_pool(name="ps", bufs=3, space="PSUM"))
        psum2 = ctx.enter_context(tc.tile_pool(name="ps2", bufs=3, space="PSUM"))
        psum3 = ctx.enter_context(tc.tile_pool(name="ps3", bufs=2, space="PSUM"))

        # PE warm-up: the HAM clock gate holds the array at low clock until
        # it has been busy a while; burn dummy matmuls during the initial
        # DMA wait so real matmuls start at a higher clock.
        dummy = const.tile([P, 128], bf16)
        nc.gpsimd.memset(dummy[:], 0.0)
        dummy2 = const.tile([P, 512], bf16)
        nc.gpsimd.memset(dummy2[:], 0.0)
        dps = psum.tile([64, 512], f32, tag="ps")
        for i in range(WARMUP):
            nc.tensor.matmul(dps[:, :128], dummy[:, :64], dummy[:, :128],
                             start=True, stop=True)
        # longer dummies sized so the warmup ends right as the first x0/wsm
        # pieces land (~11.5us); each is ~427ns at the ramp clock
        for i in range(5):
            nc.tensor.matmul(dps[:], dummy2[:, :64], dummy2[:],
                             start=True, stop=True)

        # ---- head DMA issue, spread across the 3 trigger queues ----
        # each trigger costs ~610ns of queue time, so the first L1 chain's
        # operands (x0[a0] k-pieces, wsm0) take the FIRST two slots of
        # every queue; everything else follows by deadline
        a0 = agent_list[0][1]
        x0_first = xpool.tile([P, KD, NB], bf16, tag="x0")
        wsm = [const.tile([P, KD, P], bf16, tag=f"wsm{ms}", name=f"wsm{ms}")
               for ms in range(MH)]
        # a transfer's descriptors ride ONE DMA ring (~23-46GB/s), so the
        # first chain's x0 k-pieces are split into halves that pull on
        # parallel rings; queue slot order is by consumption deadline
        HB = NB // 2
        q0, q1, q2 = nc.sync, nc.gpsimd, nc.scalar
        q0.dma_start(x0_first[:, 0, :HB], x0p[a0][:, 0, :HB])
        q1.dma_start(x0_first[:, 0, HB:], x0p[a0][:, 0, HB:])
        q2.dma_start(x0_first[:, 1, :HB], x0p[a0][:, 1, :HB])
        q0.dma_start(x0_first[:, 1, HB:], x0p[a0][:, 1, HB:])
        q1.dma_start(wsm[0][:, 0, :], wsp[:, 0, 0, :])
        q2.dma_start(wsm[0][:, 1, :], wsp[:, 0, 1, :])
        q1.dma_start(x0_first[:, 2, :HB], x0p[a0][:, 2, :HB])
        q2.dma_start(x0_first[:, 2, HB:], x0p[a0][:, 2, HB:])
        q0.dma_start(wsm[0][:, 2, :], wsp[:, 0, 2, :])
        q0.dma_start(x0_first[:, 3, :HB], x0p[a0][:, 3, :HB])
        q2.dma_start(x0_first[:, 3, HB:], x0p[a0][:, 3, HB:])
        q1.dma_start(wsm[0][:, 3, :], wsp[:, 0, 3, :])
        # wsm1/wsm2 split in k-halves so their first pieces beat the ms1/ms2
        # chains (a whole 128KB tile lands ~2.6us too late for ms1)
        KHALF = KD // 2
        q1.dma_start(wsm[1][:, :KHALF], wsp[:, 1, :KHALF])
        q0.dma_start(wsm[1][:, KHALF:], wsp[:, 1, KHALF:])
        q2.dma_start(wsm[2][:, :KHALF], wsp[:, 2, :KHALF])
        q0.dma_start(wsm[2][:, KHALF:], wsp[:, 2, KHALF:])
        q0.dma_start(wsm[3][:], wsp[:, 3])
        bs_t = const.tile([P, MH], f32)
        nc.scalar.dma_start(bs_t[:], bsp[:, :])

        # preloaded expert weights: all groups resident in SBUF
        w1all = const.tile([P, E, KH, H], bf16, name="w1all")
        w2all = const.tile([P, E, KH, NA], bf16, name="w2all")
        b1all = const.tile([P, E, MH], f32, name="b1all")
        b2all = const.tile([P, E, NA], f32, name="b2all")

        x0_tiles = {0: x0_first}

        def dma_x0(t, eng, eng2=None):
            if t >= NAG or t in x0_tiles:
                return
            a = agent_list[t][1]
            x0_t = xpool.tile([P, KD, NB], bf16, tag="x0", name=f"x0_{a}")
            if eng2 is not None:  # head: split halves across two queues
                eng.dma_start(x0_t[:, :KD // 2], x0p[a][:, :KD // 2])
                eng2.dma_start(x0_t[:, KD // 2:], x0p[a][:, KD // 2:])
            else:
                eng.dma_start(x0_t[:], x0p[a])
            x0_tiles[t] = x0_t

        dma_x0(1, nc.gpsimd, nc.scalar)
        dma_x0(2, nc.gpsimd, nc.scalar)
        # group 0's W1 is on the first-L2 critical path (~22us); halves so
        # both rings pull it in parallel
        nc.gpsimd.dma_start(w1all[:, 0, :KH // 2], w1p[0][:, :KH // 2])
        nc.scalar.dma_start(w1all[:, 0, KH // 2:], w1p[0][:, KH // 2:])
        nc.sync.dma_start(b1all[:], b1p[:, :])
        nc.sync.dma_start(b2all[:], b2p[:, :])
        nc.sync.dma_start(w2all[:], w2p[:, :])
        # W1 for groups 1..E-1 (4MB total) is staggered into the agent
        # loop — one group per pair — so the head's x0 transfers don't
        # fight it for DMA-ring bandwidth; group s is first needed around
        # 27us*s with the trigger fired ~20us*s, transfer ~5us.
        w1_pending = list(range(1, E))

        # steady-state x0 prefetch: one trigger, rotating queues (sync is
        # reserved for output stores)
        dma_engines = [nc.gpsimd, nc.scalar]

        def emit_l1(a, x0_t):
            x1_t = x1pool.tile([P, MH, NB], bf16, tag="x1", name=f"x1_{a}")
            for ms in range(MH):
                ps1 = psum.tile([P, NB], f32, tag="ps", name=f"ps1_{a}_{ms}")
                for ks in range(KD):
                    nc.tensor.matmul(
                        ps1[:], wsm[ms][:, ks, :], x0_t[:, ks, :],
                        start=(ks == 0), stop=(ks == KD - 1),
                    )
                if ms % 2:
                    nc.vector.tensor_scalar(
                        x1_t[:, ms, :], ps1[:], bs_t[:, ms:ms + 1], 0.0,
                        mybir.AluOpType.add, mybir.AluOpType.max)
                else:
                    nc.scalar.activation(x1_t[:, ms, :], ps1[:], Relu,
                                         bias=bs_t[:, ms:ms + 1])
            return x1_t

        def emit_l2(a, x1_t, s, pool=None):
            # pool/tag must match an existing allocation: a new tag in a
            # pool ADDS bufs x size to it (PSUM has no headroom)
            pool, ptag = (pool, "ps") if pool is not None else (psum2, "ps2")
            h_t = hpool.tile([P, MH, NB], bf16, tag="h", name=f"h_{a}")
            for ms in range(MH):
                ps2 = pool.tile([P, NB], f32, tag=ptag, name=f"ps2_{a}_{ms}")
                for ks in range(KH):
                    nc.tensor.matmul(
                        ps2[:],
                        w1all[:, s, ks, ms * P:(ms + 1) * P],
                        x1_t[:, ks, :],
                        start=(ks == 0), stop=(ks == KH - 1),
                    )
                if ms % 2:
                    nc.vector.tensor_scalar(
                        h_t[:, ms, :], ps2[:], b1all[:, s, ms:ms + 1], 0.0,
                        mybir.AluOpType.add, mybir.AluOpType.max)
                else:
                    nc.scalar.activation(h_t[:, ms, :], ps2[:], Relu,
                                         bias=b1all[:, s, ms:ms + 1])
            return h_t

        def emit_l3_tail(a, h_t, s):
            # batch-stationary: stationary = h [128k, 128b-block], moving =
            # W2 [128k, NA]; out [128b, NA] accumulated over KH k-tiles.
            ps3 = psum3.tile([P, JB, NA], f32, tag="ps3", name=f"ps3_{a}")
            for jb in range(JB):
                for ks in range(KH):
                    nc.tensor.matmul(
                        ps3[:, jb, :],
                        h_t[:, ks, jb * P:(jb + 1) * P],
                        w2all[:, s, ks, :],
                        start=(ks == 0), stop=(ks == KH - 1),
                    )
            o_t = opool.tile([P, JB, NA], f32, tag="o", name=f"o_{a}")
            nc.vector.tensor_add(
                o_t[:], ps3[:],
                b2all[:, s:s + 1, :].to_broadcast((P, JB, NA)),
            )
            # store trigger deferred to the NEXT iteration: on the sync
            # queue it blocks until the DVE add completes, and anything
            # queued behind it would inherit that wait
            return (a, o_t)

        def store(a, o_t):
            nc.sync.dma_start(yt[a], o_t[:])

        # two-stage software pipeline over agent PAIRS: each round emits
        #   L1(a), L1(a+1) | L3(a-4), L3(a-3) | L2(a-2), L2(a-1)
        pend_l2 = []     # [(a, x1_t, s)] — L1 done, L2 not yet emitted
        pend_tail = []   # [(a, h_t, s)] — L2 done, L3 deferred
        pend_store = []  # [(a, o_t)] — output computed, store not queued
        for t, (s, a) in enumerate(agent_list):
            dma_x0(t + 3, dma_engines[t % 2])
            if t % 2 == 0 and w1_pending:
                sw = w1_pending.pop(0)
                nc.sync.dma_start(w1all[:, sw], w1p[sw])
            for sa, so in pend_store:
                store(sa, so)
            pend_store = []
            x1_t = emit_l1(a, x0_tiles.pop(t))
            pend_l2.append((a, x1_t, s))
            if t % 2 == 1:
                for args in pend_tail:
                    pend_store.append(emit_l3_tail(*args))
                pend_tail = []
                while len(pend_l2) > 2:
                    pa, px1, ps_ = pend_l2.pop(0)
                    h_t = emit_l2(pa, px1, ps_)
                    pend_tail.append((pa, h_t, ps_))
        def emit_l3_final(a, h_t, s, engs):
            # tail-only variant: per-half bias add + store on separate
            # queues/rings so the final 64KB store (~2.8us on one ring)
            # doesn't serialize after the last matmul
            JH = JB // 2
            o_t = opool.tile([P, JB, NA], f32, tag="o", name=f"of_{a}")
            for half in range(2):
                ps3 = psum3.tile([P, JH, NA], f32, tag="ps3",
                                 name=f"ps3f_{a}_{half}")
                for jj in range(JH):
                    jb = half * JH + jj
                    for ks in range(KH):
                        nc.tensor.matmul(
                            ps3[:, jj, :],
                            h_t[:, ks, jb * P:(jb + 1) * P],
                            w2all[:, s, ks, :],
                            start=(ks == 0), stop=(ks == KH - 1),
                        )
                sl = slice(half * JH, (half + 1) * JH)
                nc.vector.tensor_add(
                    o_t[:, sl], ps3[:],
                    b2all[:, s:s + 1, :].to_broadcast((P, JH, NA)),
                )
                engs[half].dma_start(yt[a][:, sl], o_t[:, sl])

        # drain — both remaining L2s before both L3 tails, so the first
        # tail's activations get a full L2 phase of slack
        for sa, so in pend_store:
            store(sa, so)
        for args in pend_tail:
            store(*emit_l3_tail(*args))
        done_l2 = []
        # the L1 psum pool is idle during the drain: running the last L2
        # from it avoids a psum2 bank stall (its banks are still draining
        # through the L2 activations of the previous agent)
        for i, (pa, px1, ps_) in enumerate(pend_l2):
            pool = psum if i == len(pend_l2) - 1 else None
            done_l2.append((pa, emit_l2(pa, px1, ps_, pool=pool), ps_))
        tail_engs = [(nc.sync, nc.gpsimd), (nc.scalar, nc.gpsimd)]
        for i, (pa, h_t, ps_) in enumerate(done_l2):
            emit_l3_final(pa, h_t, ps_, tail_engs[i % 2])

    nc.compile()
    return nc


def kernel(x0, W_shared, b_shared, W1, b1, W2, b2, route,
           _trace=False, _tmpdir=None):
    import ml_dtypes
    from concourse.bass_utils import run_bass_kernel_spmd

    bf16 = ml_dtypes.bfloat16
    x0 = np.asarray(x0, dtype=np.float32)
    W_shared = np.asarray(W_shared, dtype=np.float32)
    b_shared = np.asarray(b_shared, dtype=np.float32)
    W1 = np.asarray(W1, dtype=np.float32)
    b1 = np.asarray(b1, dtype=np.float32)
    W2 = np.asarray(W2, dtype=np.float32)
    b2 = np.asarray(b2, dtype=np.float32)
    route = np.asarray(route)

    B, A, D = x0.shape
    H = W_shared.shape[1]
    NA = W2.shape[2]
    Bl = B // N_CORES
    KD, KH, MH = D // P, H // P, H // P
    JB = Bl // P

    experts, inv = np.unique(route, return_inverse=True)
    groups = tuple(tuple(np.where(inv == s)[0].tolist())
                   for s in range(len(experts)))
    E = len(experts)

    key = (B, A, D, H, NA, groups)
    nc = _cache.get(key)
    if nc is None:
        nc = _build(A, D, H, NA, Bl, groups)
        _cache[key] = nc

    # host-side shard + transpose to partition-major bf16 layouts
    # x0p[c][a, p, ks, b] = x0[c*Bl + b, a, ks*P + p]
    x0p = np.ascontiguousarray(
        x0.astype(bf16).reshape(N_CORES, Bl, A, KD, P)
        .transpose(0, 2, 4, 3, 1))
    # wsp[p, ms, ks, q] = Ws[ks*P + p, ms*P + q]
    wsp = np.ascontiguousarray(
        W_shared.astype(bf16).reshape(KD, P, MH, P).transpose(1, 2, 0, 3))
    bsp = np.ascontiguousarray(b_shared.reshape(MH, P).T)
    # w1p[s, p, ks, h] = W1[experts[s], ks*P + p, h]
    w1p = np.ascontiguousarray(
        W1[experts].astype(bf16).reshape(E, KH, P, H).transpose(0, 2, 1, 3))
    # b1p[p, s, ms] = b1[experts[s], ms*P + p]
    b1p = np.ascontiguousarray(
        b1[experts].reshape(E, MH, P).transpose(2, 0, 1))
    # w2p[p, s, ks, n] = W2[experts[s], ks*P + p, n]
    w2p = np.ascontiguousarray(
        W2[experts].astype(bf16).reshape(E, KH, P, NA).transpose(2, 0, 1, 3))
    b2p = np.ascontiguousarray(
        np.broadcast_to(b2[experts][None, :, :], (P, E, NA)))

    in_maps = [
        dict(x0p=x0p[c], wsp=wsp, bsp=bsp,
             w1p=w1p, b1p=b1p, w2p=w2p, b2p=b2p)
        for c in range(N_CORES)
    ]
    # the axon-proxied runtime occasionally reports a transient
    # "device unrecoverable" right after another process released the
    # cores; a short-delay retry recovers it
    import time
    last_err = None
    for attempt in range(3):
        try:
            res = run_bass_kernel_spmd(nc, in_maps,
                                       core_ids=list(range(N_CORES)),
                                       trace=_trace, tmpdir=_tmpdir)
            break
        except Exception as e:  # noqa: BLE001
            last_err = e
            time.sleep(5.0 * (attempt + 1))
    else:
        raise last_err
    kernel.last_exec_time_ns = res.exec_time_ns
    # yt[c][a, p, j, n] = y[c*Bl + j*P + p, a, n]
    yt = np.stack([res.results[c]["yt"] for c in range(N_CORES)])
    y = np.ascontiguousarray(
        yt.transpose(0, 3, 2, 1, 4)).reshape(B, A, NA)
    return y
